# revision 35
# baseline (speedup 1.0000x reference)
"""Bayesian multi-task MLP (moe_routing) — Trainium2 Bass/Tile kernel.

Reference computation (per forward):
    w   = mu + exp(ls) * eps                    (Bayesian reparameterization)
    h   = relu(x @ w0.T + b0)                   [4096, 2048]
    h   = relu(h @ wi.T + bi)   for i in 0..2   (2048 -> 2048)
    out = (h @ hw[t].T + hb[t]) routed per-sample by task id   [4096, 10]

Distribution: pure data-parallel over the batch — each of the 8 cores gets
512 rows of x/task and a full replica of the (device-prepared) Bayesian
params.  No collectives.

v3 changes over the 234us baseline (which co-saturated PE+DVE+ACT):
  * the +mu add rides the mu DMA itself (accum_op=add into the noise
    tile), so DVE only does the sigma*eps multiply.  The accum DMAs are
    issued from the gpsimd (SWDGE) queue so their wait-on-mul never
    head-of-line-blocks the ls/eps prefetch stream on sync.
  * ACT exp evaluates sigma directly (the -6 shift rides the activation
    bias as a [128,1] const tile).
  * weights pre-tiled on host to [half, 128, kt, 1024] so each 2-k-tile
    group DMA is one contiguous run per partition (128 descriptors).
  * startup: first-group DMAs fan across sync/scalar queues in parallel;
    bias/head/task constants live off the critical sync queue; first two
    groups use a plain mu DMA + DVE add to shorten the first-MM chain.
  * tail: onehot routing masks precomputed mid-kernel; one gathered
    output DMA instead of four.

All math (exp, mul, add, matmuls, bias, relu, routing select) runs
on-device; host prep is layout/dtype only.
"""

import numpy as np

import concourse.bacc as bacc
import concourse.mybir as mybir
from concourse.bass_utils import run_bass_kernel_spmd
from concourse.tile import TileContext

NCORES = 8
B, IN, H, OUT, T, NL = 4096, 1024, 2048, 10, 10, 3
BC = B // NCORES           # batch rows per core = 512
TO = T * OUT               # flattened head outputs = 100
KT_IN = IN // 128          # k-tiles in layer 0 = 8
KT_H = H // 128            # k-tiles in hidden layers = 16
OT_HALF = 8                # out-tiles per half (8 PSUM banks)

F16 = mybir.dt.float16
F32 = mybir.dt.float32
F8 = mybir.dt.float8e4
ALU = mybir.AluOpType
ACTF = mybir.ActivationFunctionType

# group sizes (k-tiles) per half: small lead-in for layer-0 half 0
GS_L0_FIRST = [1, 1, 2, 2, 2]
GS_L0 = [2, 2, 2, 2]
GS_H = [4, 4, 4, 4]
GMAX = 4


def build_nc():
    nc = bacc.Bacc(trn_type="TRN2")

    # ---- per-core DRAM I/O ----
    xT = nc.dram_tensor("xT", [128, KT_IN, BC], F16, kind="ExternalInput")
    # weights pre-tiled [2(half), 128, kt, 1024] (layer0) / [NL, 2, 128, kt, 1024]
    muT0 = nc.dram_tensor("muT0", [2, 128, KT_IN, 1024], F16, kind="ExternalInput")
    lsT0 = nc.dram_tensor("lsT0", [2, 128, KT_IN, 1024], F8, kind="ExternalInput")
    epsT0 = nc.dram_tensor("epsT0", [2, 128, KT_IN, 1024], F8, kind="ExternalInput")
    muT = nc.dram_tensor("muT", [NL, 2, 128, KT_H, 1024], F16, kind="ExternalInput")
    lsT = nc.dram_tensor("lsT", [NL, 2, 128, KT_H, 1024], F8, kind="ExternalInput")
    epsT = nc.dram_tensor("epsT", [NL, 2, 128, KT_H, 1024], F8, kind="ExternalInput")
    # head weights pre-tiled on host to [128, k, to] (contiguous per partition)
    muhT = nc.dram_tensor("muhT", [128, KT_H, TO], F16, kind="ExternalInput")
    lshT = nc.dram_tensor("lshT", [128, KT_H, TO], F16, kind="ExternalInput")
    epshT = nc.dram_tensor("epshT", [128, KT_H, TO], F16, kind="ExternalInput")
    # biases for the 4 dense layers, pre-tiled [128, layer, otile], fp32
    mub = nc.dram_tensor("mub", [128, NL + 1, KT_H], F32, kind="ExternalInput")
    lsb = nc.dram_tensor("lsb", [128, NL + 1, KT_H], F32, kind="ExternalInput")
    epsb = nc.dram_tensor("epsb", [128, NL + 1, KT_H], F32, kind="ExternalInput")
    muhb = nc.dram_tensor("muhb", [1, TO], F32, kind="ExternalInput")
    lshb = nc.dram_tensor("lshb", [1, TO], F32, kind="ExternalInput")
    epshb = nc.dram_tensor("epshb", [1, TO], F32, kind="ExternalInput")
    taskf = nc.dram_tensor("taskf", [128, BC // 128], F32, kind="ExternalInput")
    # fp16 copy of eps k-tiles 0..3 per hidden-layer half: buys the DVE 2x
    # tensor_tensor mode for one group per half (fp8 operands force 1x)
    epsT16 = nc.dram_tensor(
        "epsT16", [NL, 2, 128, GMAX, 1024], F16, kind="ExternalInput")
    # output stays in the sbuf-native [128, m, o] layout; host untiles it
    out = nc.dram_tensor("out", [128, BC // 128, OUT], F32, kind="ExternalOutput")

    with TileContext(nc) as tc:
        with (
            tc.tile_pool(name="const", bufs=1) as cpool,
            tc.tile_pool(name="wdma", bufs=5) as dpool,
            tc.tile_pool(name="wprep", bufs=3) as xpool,
            tc.tile_pool(name="hbuf", bufs=3) as hpool,
            tc.tile_pool(name="sel", bufs=4) as spool,
            tc.tile_pool(name="psum", bufs=8, space="PSUM") as ppool,
        ):
            # ---- startup: first group (layer0 half0 k=0) fans across queues
            g0 = 1  # k-tiles in the first group
            ls_g0 = dpool.tile([128, GMAX, 1024], F8, tag="ls", name="ls_g")
            eps_g0 = dpool.tile([128, GMAX, 1024], F8, tag="eps", name="eps_g")
            mu_g0 = dpool.tile([128, GMAX, 1024], F16, tag="mu",
                                  name="mu_g")
            nc.sync.dma_start(out=ls_g0[:, :g0, :], in_=lsT0.ap()[0][:, 0:g0, :])
            nc.scalar.dma_start(out=eps_g0[:, :g0, :], in_=epsT0.ap()[0][:, 0:g0, :])
            nc.sync.dma_start(out=mu_g0[:, :g0, :], in_=muT0.ap()[0][:, 0:g0, :])

            hT_x = hpool.tile([128, KT_IN, BC], F16, tag="hT")
            nc.gpsimd.dma_start(out=hT_x[:, 0:1, :], in_=xT.ap()[:, 0:1, :])

            # constant DMAs are emitted mid-layer-0 (post_g2 hook below) so
            # their descriptors never sit ahead of the early weight groups
            bias_mu = cpool.tile([128, NL + 1, KT_H], F32, name="bias_mu")
            bias_ls = cpool.tile([128, NL + 1, KT_H], F32, name="bias_ls")
            bias_eps = cpool.tile([128, NL + 1, KT_H], F32, name="bias_eps")
            taskt = cpool.tile([128, BC // 128], F32)

            def emit_const_dmas():
                nc.sync.dma_start(out=bias_mu, in_=mub.ap())
                nc.sync.dma_start(out=bias_ls, in_=lsb.ap())
                nc.sync.dma_start(out=bias_eps, in_=epsb.ap())
                nc.sync.dma_start(out=taskt, in_=taskf.ap())

            neg6 = cpool.tile([128, 1], F32, name="neg6")
            nc.vector.memset(neg6, -6.0)
            # dummy activation preloads the exp LUT set before the first
            # weight slab lands, keeping ACT_TABLE_LOAD off the critical path
            warm = cpool.tile([128, 1], F32, name="warm")
            nc.scalar.activation(out=warm, in_=neg6, func=ACTF.Exp)

            # warm the PE / HAM clock gate with throwaway matmuls while the
            # first weight group is still in flight (~3.4us of activity flips
            # the clock gate from 1.2 to 2.4 GHz before the real stream)
            wz_l = cpool.tile([128, 128], F16, name="wz_l")
            wz_r = cpool.tile([128, BC], F16, name="wz_r")
            nc.vector.memset(wz_l, 0.0)
            nc.vector.memset(wz_r, 0.0)
            warm_ps = ppool.tile([128, BC], F32, tag="mm", name="warm_ps")
            for _ in range(12):
                nc.tensor.matmul(warm_ps, lhsT=wz_l, rhs=wz_r, start=True,
                                 stop=True)

            # dense-layer bias b = mu + exp(ls)*eps, built lazily so the ACT
            # queue reaches the first weight exp without waiting on bias DMAs
            _bias_cache = []

            def get_bias():
                if not _bias_cache:
                    b = cpool.tile([128, NL + 1, KT_H], F32, name="bias")
                    nc.scalar.activation(out=b, in_=bias_ls, func=ACTF.Exp)
                    nc.vector.tensor_mul(b, b, bias_eps)
                    nc.vector.tensor_add(b, b, bias_mu)
                    _bias_cache.append(b)
                return _bias_cache[0]

            flat = lambda g, n: g.rearrange("p a b -> p (a b)")[:, :n]

            def ff_layer(hT_in, kt, gsizes_half0, mu_ap, ls_ap, eps_ap, bias_l,
                         first=False, eps16_ap=None, scalar_g0_halves=()):
                """hT_out[o, b] = relu(w @ hT_in + b) over 2 halves of ocols."""
                hT_out = hpool.tile([128, KT_H, BC], F16, tag="hT", name="hT_out")
                for half in range(2):
                    psums = []
                    for o8 in range(OT_HALF):
                        ps = ppool.tile([128, BC], F32, tag="mm", name="ps")
                        psums.append(ps)
                    gsizes = gsizes_half0 if half == 0 else (
                        GS_L0 if kt == KT_IN else GS_H)
                    k0 = 0
                    for gi, gs in enumerate(gsizes):
                        n = gs * 1024
                        # early halves' first groups ride the scalar queue so
                        # descriptor generation overlaps the sync stream
                        if first and half == 0 and gi == 1:
                            dq = nc.scalar
                        elif gi == 0 and half in scalar_g0_halves:
                            dq = nc.scalar
                        else:
                            dq = nc.sync
                        use16 = eps16_ap is not None and gi == 0
                        if first and half == 0 and gi == 0:
                            ls_g, eps_g, mu_g = ls_g0, eps_g0, mu_g0
                        else:
                            ls_g = dpool.tile([128, GMAX, 1024], F8, tag="ls", name="ls_g")
                            mu_g = dpool.tile([128, GMAX, 1024], F16, tag="mu",
                                              name="mu_g")
                            dq.dma_start(
                                out=ls_g[:, :gs, :], in_=ls_ap[half][:, k0:k0 + gs, :])
                            if use16:
                                eps_g = dpool.tile([128, GMAX, 1024], F16,
                                                   tag="e16", bufs=2, name="e16_g")
                                dq.dma_start(
                                    out=eps_g[:, :gs, :], in_=eps16_ap[half])
                            else:
                                eps_g = dpool.tile([128, GMAX, 1024], F8,
                                                   tag="eps", name="eps_g")
                                dq.dma_start(
                                    out=eps_g[:, :gs, :],
                                    in_=eps_ap[half][:, k0:k0 + gs, :])
                            dq.dma_start(
                                out=mu_g[:, :gs, :],
                                in_=mu_ap[half][:, k0:k0 + gs, :])
                        if first and half == 0 and gi == 2:
                            # x k-tiles 2..7 must be in flight before this
                            # group's matmuls (they read hT_x[:, 2:4, :])
                            nc.sync.dma_start(
                                out=hT_x[:, 2:KT_IN, :], in_=xT.ap()[:, 2:KT_IN, :])
                        # sigma = exp((ls+6) - 6): host ships ls+6 in fp8; the
                        # ACT bias undoes the shift inside the LUT evaluation.
                        # exp runs per 2-slab pair (amortizes ACT init/dispatch,
                        # ACT was 95% busy at slab granularity); mul/add stay
                        # per k-slab [128, 1024] — fine-grained units pipeline
                        # across engines better than one group op.
                        t_g = xpool.tile([128, GMAX, 1024], F16, tag="t",
                                         bufs=3, name="t_g")
                        for h2 in range(0, gs, 2):
                            p2 = min(2, gs - h2)
                            nc.scalar.activation(
                                out=t_g[:, h2:h2 + p2, :].rearrange(
                                    "p a b -> p (a b)"),
                                in_=ls_g[:, h2:h2 + p2, :].rearrange(
                                    "p a b -> p (a b)"),
                                func=ACTF.Exp, bias=neg6)
                        for ks in range(gs):
                            k = k0 + ks
                            w_s = xpool.tile([128, 1024], F16, tag="w", bufs=6,
                                             name="w_s")
                            nc.vector.tensor_mul(
                                w_s, t_g[:, ks, :], eps_g[:, ks, :])
                            nc.vector.tensor_add(w_s, w_s, mu_g[:, ks, :])
                            for o8 in range(OT_HALF):
                                nc.tensor.matmul(
                                    psums[o8],
                                    lhsT=w_s[:, o8 * 128:(o8 + 1) * 128],
                                    rhs=hT_in[:, k, :],
                                    start=(k == 0),
                                    stop=(k == kt - 1),
                                )
                        k0 += gs
                        if first and half == 0 and gi == 0:
                            # x k-tile 1 right behind the first group
                            nc.sync.dma_start(
                                out=hT_x[:, 1:2, :], in_=xT.ap()[:, 1:2, :])
                        if first and half == 0 and gi == 3:
                            emit_const_dmas()
                    for o8 in range(OT_HALF):
                        o = half * OT_HALF + o8
                        nc.scalar.activation(
                            out=hT_out[:, o, :],
                            in_=psums[o8],
                            func=ACTF.Relu,
                            bias=get_bias()[:, bias_l, o:o + 1],
                        )
                return hT_out

            cur = ff_layer(hT_x, KT_IN, GS_L0_FIRST, muT0.ap(), lsT0.ap(),
                           epsT0.ap(), 0, first=True, scalar_g0_halves=(1,))

            # routing constants (cheap; built while layer 1 runs)
            iota10 = cpool.tile([128, T], mybir.dt.int32)
            nc.gpsimd.iota(iota10, [[1, T]], base=0, channel_multiplier=0)
            iota10f = cpool.tile([128, T], F32)
            nc.vector.tensor_copy(out=iota10f, in_=iota10)
            onehots = []
            for m in range(BC // 128):
                oh = cpool.tile([128, T], F32, name="onehot")
                nc.vector.tensor_single_scalar(
                    out=oh, in_=iota10f, scalar=taskt[:, m:m + 1], op=ALU.is_equal
                )
                onehots.append(oh)

            cur = ff_layer(cur, KT_H, GS_H, muT.ap()[0], lsT.ap()[0],
                           epsT.ap()[0], 1, eps16_ap=epsT16.ap()[0],
                           scalar_g0_halves=(0,))

            # head weight/bias DMAs ride the scalar queue mid-kernel
            hb_mu = cpool.tile([1, TO], F32)
            hb_ls = cpool.tile([1, TO], F32)
            hb_eps = cpool.tile([1, TO], F32)
            nc.scalar.dma_start(out=hb_mu, in_=muhb.ap())
            nc.scalar.dma_start(out=hb_ls, in_=lshb.ap())
            nc.scalar.dma_start(out=hb_eps, in_=epshb.ap())
            wh_mu = cpool.tile([128, KT_H, TO], F16)
            wh_ls = cpool.tile([128, KT_H, TO], F16)
            wh_eps = cpool.tile([128, KT_H, TO], F16)
            nc.scalar.dma_start(out=wh_mu, in_=muhT.ap())
            nc.scalar.dma_start(out=wh_ls, in_=lshT.ap())
            nc.scalar.dma_start(out=wh_eps, in_=epshT.ap())

            cur = ff_layer(cur, KT_H, GS_H, muT.ap()[1], lsT.ap()[1],
                           epsT.ap()[1], 2, eps16_ap=epsT16.ap()[1])

            # head constants: w_h = mu + exp(ls)*eps (fp16), hb folded via PE
            hb_f = cpool.tile([1, TO], F32)
            nc.scalar.activation(out=hb_f, in_=hb_ls, func=ACTF.Exp)
            nc.vector.tensor_mul(hb_f, hb_f, hb_eps)
            nc.vector.tensor_add(hb_f, hb_f, hb_mu)
            hb16 = cpool.tile([1, TO], F16)
            nc.vector.tensor_copy(out=hb16, in_=hb_f)
            ones1 = cpool.tile([1, 128], F16)
            nc.vector.memset(ones1, 1.0)
            whT = cpool.tile([128, KT_H, TO], F16)
            nc.scalar.activation(out=whT, in_=wh_ls, func=ACTF.Exp)
            nc.vector.tensor_mul(whT, whT, wh_eps)
            nc.vector.tensor_add(whT, whT, wh_mu)

            # ---- last hidden layer ----
            cur = ff_layer(cur, KT_H, GS_H, muT.ap()[2], lsT.ap()[2],
                           epsT.ap()[2], 3, eps16_ap=epsT16.ap()[2])

            # ---- heads + routing select; one gathered output DMA ----
            outm_all = cpool.tile([128, BC // 128, OUT], F32, name="outm_all")
            for m in range(BC // 128):
                ps = ppool.tile([128, TO], F32, tag="mm", name="ps_head")
                for k in range(KT_H):
                    nc.tensor.matmul(
                        ps,
                        lhsT=cur[:, k, m * 128:(m + 1) * 128],
                        rhs=whT[:, k, :],
                        start=(k == 0),
                        stop=False,
                    )
                nc.tensor.matmul(
                    ps, lhsT=ones1[:1, :], rhs=hb16[:1, :], start=False, stop=True
                )
                masked = spool.tile([128, OUT, T], F32, name="masked")
                ps_v = ps.rearrange("p (t o) -> p o t", t=T)
                oh_v = onehots[m].unsqueeze(1).broadcast_to([128, OUT, T])
                nc.vector.tensor_tensor(masked, ps_v, oh_v, ALU.mult)
                nc.vector.tensor_reduce(
                    out=outm_all[:, m, :], in_=masked, axis=mybir.AxisListType.X,
                    op=ALU.add,
                )
            nc.sync.dma_start(out=out.ap(), in_=outm_all)

    nc.finalize()
    return nc


_CACHE = {}


def _prep_host(inputs):
    """Layout/dtype prep + batch sharding. Returns list of per-core in_maps."""
    import ml_dtypes

    f16 = np.float16
    f8 = ml_dtypes.float8_e4m3fn

    def bias_tile(b0, b):  # [4, H] -> [128, 4, 16]
        arr = np.concatenate([b0[None], b], 0).astype(np.float32)
        return np.ascontiguousarray(arr.reshape(NL + 1, KT_H, 128).transpose(2, 0, 1))

    def head_tile(a):  # [T, OUT, H] -> headT [H, TO] -> [128, 16, TO]
        aT = a.reshape(TO, H).astype(f16).T
        return np.ascontiguousarray(aT.reshape(KT_H, 128, TO).transpose(1, 0, 2))

    def wtile0(a):  # [in=1024, out=2048] -> [2, 128, 8, 1024]
        return np.ascontiguousarray(
            a.reshape(KT_IN, 128, 2, 1024).transpose(2, 1, 0, 3))

    def wtileh(a):  # [NL, in=2048, out=2048] -> [NL, 2, 128, 16, 1024]
        return np.ascontiguousarray(
            a.reshape(NL, KT_H, 128, 2, 1024).transpose(0, 3, 2, 1, 4))

    shared = {
        "muT0": wtile0(inputs["mu_w0"].astype(f16).T),
        "lsT0": wtile0((inputs["ls_w0"].T + 6.0).astype(f8)),
        "epsT0": wtile0(inputs["eps_w0"].T.astype(f8)),
        "muT": wtileh(inputs["mu_w"].astype(f16).transpose(0, 2, 1)),
        "lsT": wtileh((inputs["ls_w"].transpose(0, 2, 1) + 6.0).astype(f8)),
        "epsT": wtileh(inputs["eps_w"].transpose(0, 2, 1).astype(f8)),
        "epsT16": np.ascontiguousarray(
            wtileh(inputs["eps_w"].transpose(0, 2, 1).astype(f16))[:, :, :, :GMAX, :]),
        "muhT": head_tile(inputs["mu_hw"]),
        "lshT": head_tile(inputs["ls_hw"]),
        "epshT": head_tile(inputs["eps_hw"]),
        "mub": bias_tile(inputs["mu_b0"], inputs["mu_b"]),
        "lsb": bias_tile(inputs["ls_b0"], inputs["ls_b"]),
        "epsb": bias_tile(inputs["eps_b0"], inputs["eps_b"]),
        "muhb": inputs["mu_hb"].reshape(1, TO).astype(np.float32),
        "lshb": inputs["ls_hb"].reshape(1, TO).astype(np.float32),
        "epshb": inputs["eps_hb"].reshape(1, TO).astype(np.float32),
    }
    xTf = inputs["x"].astype(f16).T  # [IN, B]
    task = inputs["task"].astype(np.float32)
    in_maps = []
    for c in range(NCORES):
        m = dict(shared)
        xc = xTf[:, c * BC:(c + 1) * BC]  # [IN, BC]
        m["xT"] = np.ascontiguousarray(xc.reshape(KT_IN, 128, BC).transpose(1, 0, 2))
        m["taskf"] = np.ascontiguousarray(
            task[c * BC:(c + 1) * BC].reshape(BC // 128, 128).T
        )
        in_maps.append(m)
    return in_maps


def kernel(**inputs):
    inputs = {k: np.asarray(v) for k, v in inputs.items()}
    if "nc" not in _CACHE:
        _CACHE["nc"] = build_nc()
    nc = _CACHE["nc"]
    in_maps = _prep_host(inputs)
    res = run_bass_kernel_spmd(nc, in_maps, core_ids=list(range(NCORES)))
    # untile [128, m, o] -> [BC, o] per core, then concat over cores
    out = np.concatenate(
        [
            res.results[c]["out"].transpose(1, 0, 2).reshape(BC, OUT)
            for c in range(NCORES)
        ],
        axis=0,
    )
    return out.astype(np.float32)


if __name__ == "__main__":
    nc = build_nc()
    print("built ok")


# revision 37
# speedup vs baseline: 1.2195x; 1.2195x over previous
"""Bayesian multi-task MLP (moe_routing) — Trainium2 Bass/Tile kernel.

Reference computation (per forward):
    w   = mu + exp(ls) * eps                    (Bayesian reparameterization)
    h   = relu(x @ w0.T + b0)                   [4096, 2048]
    h   = relu(h @ wi.T + bi)   for i in 0..2   (2048 -> 2048)
    out = (h @ hw[t].T + hb[t]) routed per-sample by task id   [4096, 10]

Distribution: pure data-parallel over the batch — each of the 8 cores gets
512 rows of x/task and a full replica of the (device-prepared) Bayesian
params.  No collectives.

v3 changes over the 234us baseline (which co-saturated PE+DVE+ACT):
  * the +mu add rides the mu DMA itself (accum_op=add into the noise
    tile), so DVE only does the sigma*eps multiply.  The accum DMAs are
    issued from the gpsimd (SWDGE) queue so their wait-on-mul never
    head-of-line-blocks the ls/eps prefetch stream on sync.
  * ACT exp evaluates sigma directly (the -6 shift rides the activation
    bias as a [128,1] const tile).
  * weights pre-tiled on host to [half, 128, kt, 1024] so each 2-k-tile
    group DMA is one contiguous run per partition (128 descriptors).
  * startup: first-group DMAs fan across sync/scalar queues in parallel;
    bias/head/task constants live off the critical sync queue; first two
    groups use a plain mu DMA + DVE add to shorten the first-MM chain.
  * tail: onehot routing masks precomputed mid-kernel; one gathered
    output DMA instead of four.

All math (exp, mul, add, matmuls, bias, relu, routing select) runs
on-device; host prep is layout/dtype only.
"""

import numpy as np

import concourse.bacc as bacc
import concourse.mybir as mybir
from concourse.bass_utils import run_bass_kernel_spmd
from concourse.tile import TileContext

NCORES = 8
B, IN, H, OUT, T, NL = 4096, 1024, 2048, 10, 10, 3
BC = B // NCORES           # batch rows per core = 512
TO = T * OUT               # flattened head outputs = 100
KT_IN = IN // 128          # k-tiles in layer 0 = 8
KT_H = H // 128            # k-tiles in hidden layers = 16
OT_HALF = 8                # out-tiles per half (8 PSUM banks)

F16 = mybir.dt.float16
F32 = mybir.dt.float32
F8 = mybir.dt.float8e4
ALU = mybir.AluOpType
ACTF = mybir.ActivationFunctionType

# group sizes (k-tiles) per half: small lead-in for layer-0 half 0
GS_L0_FIRST = [1, 1, 2, 4]
GS_L0 = [4, 4]
GS_H = [4, 4, 4, 4]
GMAX = 4


def build_nc():
    nc = bacc.Bacc(trn_type="TRN2")

    # ---- per-core DRAM I/O ----
    xT = nc.dram_tensor("xT", [128, KT_IN, BC], F16, kind="ExternalInput")
    # weights pre-tiled [2(half), 128, kt, 1024] (layer0) / [NL, 2, 128, kt, 1024]
    muT0 = nc.dram_tensor("muT0", [2, 128, KT_IN, 1024], F16, kind="ExternalInput")
    lsT0 = nc.dram_tensor("lsT0", [2, 128, KT_IN, 1024], F8, kind="ExternalInput")
    epsT0 = nc.dram_tensor("epsT0", [2, 128, KT_IN, 1024], F8, kind="ExternalInput")
    muT = nc.dram_tensor("muT", [NL, 2, 128, KT_H, 1024], F16, kind="ExternalInput")
    lsT = nc.dram_tensor("lsT", [NL, 2, 128, KT_H, 1024], F8, kind="ExternalInput")
    epsT = nc.dram_tensor("epsT", [NL, 2, 128, KT_H, 1024], F8, kind="ExternalInput")
    # head weights pre-tiled on host to [128, k, to] (contiguous per partition)
    muhT = nc.dram_tensor("muhT", [128, KT_H, TO], F16, kind="ExternalInput")
    lshT = nc.dram_tensor("lshT", [128, KT_H, TO], F16, kind="ExternalInput")
    epshT = nc.dram_tensor("epshT", [128, KT_H, TO], F16, kind="ExternalInput")
    # biases for the 4 dense layers, pre-tiled [128, layer, otile], fp32
    mub = nc.dram_tensor("mub", [128, NL + 1, KT_H], F32, kind="ExternalInput")
    lsb = nc.dram_tensor("lsb", [128, NL + 1, KT_H], F32, kind="ExternalInput")
    epsb = nc.dram_tensor("epsb", [128, NL + 1, KT_H], F32, kind="ExternalInput")
    muhb = nc.dram_tensor("muhb", [1, TO], F32, kind="ExternalInput")
    lshb = nc.dram_tensor("lshb", [1, TO], F32, kind="ExternalInput")
    epshb = nc.dram_tensor("epshb", [1, TO], F32, kind="ExternalInput")
    taskf = nc.dram_tensor("taskf", [128, BC // 128], F32, kind="ExternalInput")
    # fp16 copy of eps k-tiles 0..3 per hidden-layer half: buys the DVE 2x
    # tensor_tensor mode for one group per half (fp8 operands force 1x)
    epsT16 = nc.dram_tensor(
        "epsT16", [NL, 2, 128, GMAX, 1024], F16, kind="ExternalInput")
    # output stays in the sbuf-native [128, m, o] layout; host untiles it
    out = nc.dram_tensor("out", [128, BC // 128, OUT], F32, kind="ExternalOutput")

    with TileContext(nc) as tc:
        with (
            tc.tile_pool(name="const", bufs=1) as cpool,
            tc.tile_pool(name="wdma", bufs=5) as dpool,
            tc.tile_pool(name="wprep", bufs=3) as xpool,
            tc.tile_pool(name="hbuf", bufs=3) as hpool,
            tc.tile_pool(name="sel", bufs=4) as spool,
            tc.tile_pool(name="psum", bufs=8, space="PSUM") as ppool,
        ):
            # ---- startup: first group (layer0 half0 k=0) fans across queues
            g0 = 1  # k-tiles in the first group
            ls_g0 = dpool.tile([128, GMAX, 1024], F8, tag="ls", name="ls_g")
            eps_g0 = dpool.tile([128, GMAX, 1024], F8, tag="eps", name="eps_g")
            mu_g0 = dpool.tile([128, GMAX, 1024], F16, tag="mu",
                                  name="mu_g")
            nc.sync.dma_start(out=ls_g0[:, :g0, :], in_=lsT0.ap()[0][:, 0:g0, :])
            nc.scalar.dma_start(out=eps_g0[:, :g0, :], in_=epsT0.ap()[0][:, 0:g0, :])
            nc.sync.dma_start(out=mu_g0[:, :g0, :], in_=muT0.ap()[0][:, 0:g0, :])

            hT_x = hpool.tile([128, KT_IN, BC], F16, tag="hT")
            nc.gpsimd.dma_start(out=hT_x[:, 0:1, :], in_=xT.ap()[:, 0:1, :])

            # constant DMAs are emitted mid-layer-0 (post_g2 hook below) so
            # their descriptors never sit ahead of the early weight groups
            bias_mu = cpool.tile([128, NL + 1, KT_H], F32, name="bias_mu")
            bias_ls = cpool.tile([128, NL + 1, KT_H], F32, name="bias_ls")
            bias_eps = cpool.tile([128, NL + 1, KT_H], F32, name="bias_eps")
            taskt = cpool.tile([128, BC // 128], F32)

            def emit_const_dmas():
                nc.sync.dma_start(out=bias_mu, in_=mub.ap())
                nc.sync.dma_start(out=bias_ls, in_=lsb.ap())
                nc.sync.dma_start(out=bias_eps, in_=epsb.ap())
                nc.sync.dma_start(out=taskt, in_=taskf.ap())

            neg6 = cpool.tile([128, 1], F32, name="neg6")
            nc.vector.memset(neg6, -6.0)
            # dummy activation preloads the exp LUT set before the first
            # weight slab lands, keeping ACT_TABLE_LOAD off the critical path
            warm = cpool.tile([128, 1], F32, name="warm")
            nc.scalar.activation(out=warm, in_=neg6, func=ACTF.Exp)

            # warm the PE / HAM clock gate with throwaway matmuls while the
            # first weight group is still in flight (~3.4us of activity flips
            # the clock gate from 1.2 to 2.4 GHz before the real stream)
            wz_l = cpool.tile([128, 128], F16, name="wz_l")
            wz_r = cpool.tile([128, BC], F16, name="wz_r")
            nc.vector.memset(wz_l, 0.0)
            nc.vector.memset(wz_r, 0.0)
            warm_ps = ppool.tile([128, BC], F32, tag="mm", name="warm_ps")
            for _ in range(12):
                nc.tensor.matmul(warm_ps, lhsT=wz_l, rhs=wz_r, start=True,
                                 stop=True)

            # dense-layer bias b = mu + exp(ls)*eps, built lazily so the ACT
            # queue reaches the first weight exp without waiting on bias DMAs
            _bias_cache = []

            def get_bias():
                if not _bias_cache:
                    b = cpool.tile([128, NL + 1, KT_H], F32, name="bias")
                    nc.scalar.activation(out=b, in_=bias_ls, func=ACTF.Exp)
                    nc.vector.tensor_mul(b, b, bias_eps)
                    nc.vector.tensor_add(b, b, bias_mu)
                    _bias_cache.append(b)
                return _bias_cache[0]

            flat = lambda g, n: g.rearrange("p a b -> p (a b)")[:, :n]

            def ff_layer(hT_in, kt, gsizes_half0, mu_ap, ls_ap, eps_ap, bias_l,
                         first=False, eps16_ap=None):
                """hT_out[o, b] = relu(w @ hT_in + b) over 2 halves of ocols."""
                hT_out = hpool.tile([128, KT_H, BC], F16, tag="hT", name="hT_out")
                for half in range(2):
                    psums = []
                    for o8 in range(OT_HALF):
                        ps = ppool.tile([128, BC], F32, tag="mm", name="ps")
                        psums.append(ps)
                    gsizes = gsizes_half0 if half == 0 else (
                        GS_L0 if kt == KT_IN else GS_H)
                    k0 = 0
                    for gi, gs in enumerate(gsizes):
                        n = gs * 1024
                        # group 1 of layer 0 rides the scalar queue so its
                        # transfer overlaps group 0's prep chain
                        dq = nc.scalar if (first and half == 0 and gi == 1) else nc.sync
                        use16 = eps16_ap is not None and gi == 0
                        if first and half == 0 and gi == 0:
                            ls_g, eps_g, mu_g = ls_g0, eps_g0, mu_g0
                        else:
                            ls_g = dpool.tile([128, GMAX, 1024], F8, tag="ls", name="ls_g")
                            mu_g = dpool.tile([128, GMAX, 1024], F16, tag="mu",
                                              name="mu_g")
                            dq.dma_start(
                                out=ls_g[:, :gs, :], in_=ls_ap[half][:, k0:k0 + gs, :])
                            if use16:
                                eps_g = dpool.tile([128, GMAX, 1024], F16,
                                                   tag="e16", bufs=2, name="e16_g")
                                dq.dma_start(
                                    out=eps_g[:, :gs, :], in_=eps16_ap[half])
                            else:
                                eps_g = dpool.tile([128, GMAX, 1024], F8,
                                                   tag="eps", name="eps_g")
                                dq.dma_start(
                                    out=eps_g[:, :gs, :],
                                    in_=eps_ap[half][:, k0:k0 + gs, :])
                            dq.dma_start(
                                out=mu_g[:, :gs, :],
                                in_=mu_ap[half][:, k0:k0 + gs, :])
                        if first and half == 0 and gi == 2:
                            # x k-tiles 2..7 must be in flight before this
                            # group's matmuls (they read hT_x[:, 2:4, :])
                            nc.sync.dma_start(
                                out=hT_x[:, 2:KT_IN, :], in_=xT.ap()[:, 2:KT_IN, :])
                        # sigma = exp((ls+6) - 6): host ships ls+6 in fp8; the
                        # ACT bias undoes the shift inside the LUT evaluation.
                        # exp runs per 2-slab pair (amortizes ACT init/dispatch,
                        # ACT was 95% busy at slab granularity); mul/add stay
                        # per k-slab [128, 1024] — fine-grained units pipeline
                        # across engines better than one group op.
                        t_g = xpool.tile([128, GMAX, 1024], F16, tag="t",
                                         bufs=3, name="t_g")
                        for h2 in range(0, gs, 2):
                            p2 = min(2, gs - h2)
                            nc.scalar.activation(
                                out=t_g[:, h2:h2 + p2, :].rearrange(
                                    "p a b -> p (a b)"),
                                in_=ls_g[:, h2:h2 + p2, :].rearrange(
                                    "p a b -> p (a b)"),
                                func=ACTF.Exp, bias=neg6)
                        for ks in range(gs):
                            k = k0 + ks
                            w_s = xpool.tile([128, 1024], F16, tag="w", bufs=8,
                                             name="w_s")
                            nc.vector.tensor_mul(
                                w_s, t_g[:, ks, :], eps_g[:, ks, :])
                            nc.vector.tensor_add(w_s, w_s, mu_g[:, ks, :])
                            for o8 in range(OT_HALF):
                                nc.tensor.matmul(
                                    psums[o8],
                                    lhsT=w_s[:, o8 * 128:(o8 + 1) * 128],
                                    rhs=hT_in[:, k, :],
                                    start=(k == 0),
                                    stop=(k == kt - 1),
                                )
                        k0 += gs
                        if first and half == 0 and gi == 0:
                            # x k-tile 1 right behind the first group
                            nc.sync.dma_start(
                                out=hT_x[:, 1:2, :], in_=xT.ap()[:, 1:2, :])
                        if first and half == 0 and gi == 3:
                            emit_const_dmas()
                    for o8 in range(OT_HALF):
                        o = half * OT_HALF + o8
                        nc.scalar.activation(
                            out=hT_out[:, o, :],
                            in_=psums[o8],
                            func=ACTF.Relu,
                            bias=get_bias()[:, bias_l, o:o + 1],
                        )
                return hT_out

            cur = ff_layer(hT_x, KT_IN, GS_L0_FIRST, muT0.ap(), lsT0.ap(),
                           epsT0.ap(), 0, first=True)

            # routing constants (cheap; built while layer 1 runs)
            iota10 = cpool.tile([128, T], mybir.dt.int32)
            nc.gpsimd.iota(iota10, [[1, T]], base=0, channel_multiplier=0)
            iota10f = cpool.tile([128, T], F32)
            nc.vector.tensor_copy(out=iota10f, in_=iota10)
            onehots = []
            for m in range(BC // 128):
                oh = cpool.tile([128, T], F32, name="onehot")
                nc.vector.tensor_single_scalar(
                    out=oh, in_=iota10f, scalar=taskt[:, m:m + 1], op=ALU.is_equal
                )
                onehots.append(oh)

            cur = ff_layer(cur, KT_H, GS_H, muT.ap()[0], lsT.ap()[0],
                           epsT.ap()[0], 1, eps16_ap=epsT16.ap()[0])

            # head weight/bias DMAs ride the scalar queue mid-kernel
            hb_mu = cpool.tile([1, TO], F32)
            hb_ls = cpool.tile([1, TO], F32)
            hb_eps = cpool.tile([1, TO], F32)
            nc.scalar.dma_start(out=hb_mu, in_=muhb.ap())
            nc.scalar.dma_start(out=hb_ls, in_=lshb.ap())
            nc.scalar.dma_start(out=hb_eps, in_=epshb.ap())
            wh_mu = cpool.tile([128, KT_H, TO], F16)
            wh_ls = cpool.tile([128, KT_H, TO], F16)
            wh_eps = cpool.tile([128, KT_H, TO], F16)
            nc.scalar.dma_start(out=wh_mu, in_=muhT.ap())
            nc.scalar.dma_start(out=wh_ls, in_=lshT.ap())
            nc.scalar.dma_start(out=wh_eps, in_=epshT.ap())

            cur = ff_layer(cur, KT_H, GS_H, muT.ap()[1], lsT.ap()[1],
                           epsT.ap()[1], 2, eps16_ap=epsT16.ap()[1])

            # head constants: w_h = mu + exp(ls)*eps (fp16), hb folded via PE
            hb_f = cpool.tile([1, TO], F32)
            nc.scalar.activation(out=hb_f, in_=hb_ls, func=ACTF.Exp)
            nc.vector.tensor_mul(hb_f, hb_f, hb_eps)
            nc.vector.tensor_add(hb_f, hb_f, hb_mu)
            hb16 = cpool.tile([1, TO], F16)
            nc.vector.tensor_copy(out=hb16, in_=hb_f)
            ones1 = cpool.tile([1, 128], F16)
            nc.vector.memset(ones1, 1.0)
            whT = cpool.tile([128, KT_H, TO], F16)
            nc.scalar.activation(out=whT, in_=wh_ls, func=ACTF.Exp)
            nc.vector.tensor_mul(whT, whT, wh_eps)
            nc.vector.tensor_add(whT, whT, wh_mu)

            # ---- last hidden layer ----
            cur = ff_layer(cur, KT_H, GS_H, muT.ap()[2], lsT.ap()[2],
                           epsT.ap()[2], 3, eps16_ap=epsT16.ap()[2])

            # ---- heads + routing select; one gathered output DMA ----
            outm_all = cpool.tile([128, BC // 128, OUT], F32, name="outm_all")
            for m in range(BC // 128):
                ps = ppool.tile([128, TO], F32, tag="mm", name="ps_head")
                for k in range(KT_H):
                    nc.tensor.matmul(
                        ps,
                        lhsT=cur[:, k, m * 128:(m + 1) * 128],
                        rhs=whT[:, k, :],
                        start=(k == 0),
                        stop=False,
                    )
                nc.tensor.matmul(
                    ps, lhsT=ones1[:1, :], rhs=hb16[:1, :], start=False, stop=True
                )
                masked = spool.tile([128, OUT, T], F32, name="masked")
                ps_v = ps.rearrange("p (t o) -> p o t", t=T)
                oh_v = onehots[m].unsqueeze(1).broadcast_to([128, OUT, T])
                nc.vector.tensor_tensor(masked, ps_v, oh_v, ALU.mult)
                nc.vector.tensor_reduce(
                    out=outm_all[:, m, :], in_=masked, axis=mybir.AxisListType.X,
                    op=ALU.add,
                )
            nc.sync.dma_start(out=out.ap(), in_=outm_all)

    nc.finalize()
    return nc


_CACHE = {}


def _prep_host(inputs):
    """Layout/dtype prep + batch sharding. Returns list of per-core in_maps."""
    import ml_dtypes

    f16 = np.float16
    f8 = ml_dtypes.float8_e4m3fn

    def bias_tile(b0, b):  # [4, H] -> [128, 4, 16]
        arr = np.concatenate([b0[None], b], 0).astype(np.float32)
        return np.ascontiguousarray(arr.reshape(NL + 1, KT_H, 128).transpose(2, 0, 1))

    def head_tile(a):  # [T, OUT, H] -> headT [H, TO] -> [128, 16, TO]
        aT = a.reshape(TO, H).astype(f16).T
        return np.ascontiguousarray(aT.reshape(KT_H, 128, TO).transpose(1, 0, 2))

    def wtile0(a):  # [in=1024, out=2048] -> [2, 128, 8, 1024]
        return np.ascontiguousarray(
            a.reshape(KT_IN, 128, 2, 1024).transpose(2, 1, 0, 3))

    def wtileh(a):  # [NL, in=2048, out=2048] -> [NL, 2, 128, 16, 1024]
        return np.ascontiguousarray(
            a.reshape(NL, KT_H, 128, 2, 1024).transpose(0, 3, 2, 1, 4))

    shared = {
        "muT0": wtile0(inputs["mu_w0"].astype(f16).T),
        "lsT0": wtile0((inputs["ls_w0"].T + 6.0).astype(f8)),
        "epsT0": wtile0(inputs["eps_w0"].T.astype(f8)),
        "muT": wtileh(inputs["mu_w"].astype(f16).transpose(0, 2, 1)),
        "lsT": wtileh((inputs["ls_w"].transpose(0, 2, 1) + 6.0).astype(f8)),
        "epsT": wtileh(inputs["eps_w"].transpose(0, 2, 1).astype(f8)),
        "epsT16": np.ascontiguousarray(
            wtileh(inputs["eps_w"].transpose(0, 2, 1).astype(f16))[:, :, :, :GMAX, :]),
        "muhT": head_tile(inputs["mu_hw"]),
        "lshT": head_tile(inputs["ls_hw"]),
        "epshT": head_tile(inputs["eps_hw"]),
        "mub": bias_tile(inputs["mu_b0"], inputs["mu_b"]),
        "lsb": bias_tile(inputs["ls_b0"], inputs["ls_b"]),
        "epsb": bias_tile(inputs["eps_b0"], inputs["eps_b"]),
        "muhb": inputs["mu_hb"].reshape(1, TO).astype(np.float32),
        "lshb": inputs["ls_hb"].reshape(1, TO).astype(np.float32),
        "epshb": inputs["eps_hb"].reshape(1, TO).astype(np.float32),
    }
    xTf = inputs["x"].astype(f16).T  # [IN, B]
    task = inputs["task"].astype(np.float32)
    in_maps = []
    for c in range(NCORES):
        m = dict(shared)
        xc = xTf[:, c * BC:(c + 1) * BC]  # [IN, BC]
        m["xT"] = np.ascontiguousarray(xc.reshape(KT_IN, 128, BC).transpose(1, 0, 2))
        m["taskf"] = np.ascontiguousarray(
            task[c * BC:(c + 1) * BC].reshape(BC // 128, 128).T
        )
        in_maps.append(m)
    return in_maps


def kernel(**inputs):
    inputs = {k: np.asarray(v) for k, v in inputs.items()}
    if "nc" not in _CACHE:
        _CACHE["nc"] = build_nc()
    nc = _CACHE["nc"]
    in_maps = _prep_host(inputs)
    res = run_bass_kernel_spmd(nc, in_maps, core_ids=list(range(NCORES)))
    # untile [128, m, o] -> [BC, o] per core, then concat over cores
    out = np.concatenate(
        [
            res.results[c]["out"].transpose(1, 0, 2).reshape(BC, OUT)
            for c in range(NCORES)
        ],
        axis=0,
    )
    return out.astype(np.float32)


if __name__ == "__main__":
    nc = build_nc()
    print("built ok")
